# revision 7
# baseline (speedup 1.0000x reference)
"""Trainium2 Bass kernel for LogicGatedSNN.

Computes: spikes = (spike_input @ ternarize(synapse_states).T >= 1.0)
  where ternarize(s) = +1 if s > 1, -1 if s < -1, else 0.

Strategy:
  - Data-parallel over the batch dim across 8 NeuronCores (1024 rows/core),
    weights replicated. No collectives.
  - Single-pass fp16 matmul (vs the old bf16 hi/lo double-pass): the
    ternarized weights are exact in fp16 and products with +-1 are exact,
    so the only error is the f32->fp16 cast of x (~2^-12 relative), which
    flips ~2e3 of 33.5M outputs near the threshold (rel err ~1.1e-2,
    gate 2e-2). This halves TensorE work: 2048 matmuls @ N=512/core.
  - Per core:
    * X [1024, 4096] f32 is PE-transposed (128x128 f32 tiles via identity
      matmul) into the 8 PSUM banks during the pipeline-fill window, then
      ACT-cast to fp16 k-major resident tiles (8 MB SBUF).
    * W is ternarized on the SCALAR engine: t = sign(w-1) + sign(w+1) in
      {-2,0,+2} (exactly 2x the ternary value; the DVE does the fp16 add),
      staged to DRAM per 512-row slab, reloaded k-major via the xbar
      transpose-DMA (2-byte dtype), pipelined one slab ahead of compute.
    * Matmul: psum[b 128, j 512] accumulated over 32 k-tiles, single fp16
      pass; stationary = X^T tiles [128k, 128b], moving = W''^T [128k, 512j].
    * Spike threshold on DVE straight out of PSUM at 2.0 (== current >= 1.0
      exactly, since scaling by 2 is exact in binary fp), output stored as
      fp16 0/1 in natural [b, j] layout; host casts to f32.
"""

import sys

if "/opt/trn_rl_repo" not in sys.path:
    sys.path.insert(0, "/opt/trn_rl_repo")

import numpy as np

N_CORES = 8
BATCH, IN_F, OUT_F = 8192, 4096, 4096
B_CORE = BATCH // N_CORES  # 1024

_BUILT = None


def build_bass(B, K, J, JS=512, KCH=1024, reps=1, TG=4, WBLK=None,
               wt_bufs=2, out_ring=True,
               bench_skip_xprep=False, bench_skip_wprep=False,
               bench_skip_mm=False):
    """Per-core Bass program for x:[B,K] f32, w:[J,K] f32 -> out:[B,J] fp16.

    reps > 1 repeats the whole compute (idempotent) for benchmarking via
    wall-clock deltas between builds with different reps.
    """
    from concourse import bacc
    import concourse.mybir as mybir
    import concourse.tile as tile

    f32, fp16 = mybir.dt.float32, mybir.dt.float16
    alu = mybir.AluOpType
    act = mybir.ActivationFunctionType
    P = 128
    JS = min(JS, J)
    KCH = min(KCH, K)
    BT = B // P               # batch tiles (= psum banks used)
    KT = K // P               # k tiles (partition-dim groups)
    NSLAB = J // JS           # output-feature slabs
    NKC = K // KCH            # W staging chunks along k
    assert B % P == 0 and K % P == 0 and J % JS == 0 and KT % TG == 0
    assert BT <= 8, "psum banks"
    NGRP = KT // TG           # transpose groups per b-tile

    nc = bacc.Bacc("TRN2", target_bir_lowering=False, debug=False)
    x = nc.dram_tensor("x", [B, K], f32, kind="ExternalInput")
    w = nc.dram_tensor("w", [J, K], f32, kind="ExternalInput")
    out = nc.dram_tensor("out", [B, J], fp16, kind="ExternalOutput")

    with tile.TileContext(nc) as tc:
        with (
            tc.tile_pool(name="dram", bufs=1, space="DRAM") as dpool,
            tc.tile_pool(name="xstage32", bufs=2) as xs32,
            tc.tile_pool(name="wstage32", bufs=2) as ws32,
            tc.tile_pool(name="wsign", bufs=4) as wsg,
            tc.tile_pool(name="wstage16", bufs=2) as ws16,
            tc.tile_pool(name="xtres", bufs=1) as xtres,
            tc.tile_pool(name="wtp", bufs=wt_bufs) as wtp,
            tc.tile_pool(name="ostage", bufs=8) as op,
            tc.tile_pool(name="psum", bufs=1, space="PSUM") as pp,
        ):
            if WBLK is None:
                WBLK = JS
            WBLK = min(WBLK, JS)
            assert JS % WBLK == 0
            # DRAM scratch: ternarized(x2) W. Separate tiles per row-block
            # keep RAW deps slab-granular for pipelining.
            wt_blocks = [
                dpool.tile([WBLK, K], fp16, name=f"wt_nat_r{r}")
                for r in range(J // WBLK)
            ]

            from concourse.masks import make_identity

            ident = xtres.tile([P, P], f32, name="ident")
            make_identity(nc, ident[:])
            neg1 = xtres.tile([P, 1], f32, name="neg1")
            nc.vector.memset(neg1[:], -1.0)

            odma = nc.scalar if out_ring else nc.sync

            def tern_slab_rows(j0, js):
                # t = sign(w-1) + sign(w+1) in {-2,0,+2}, fp16, for rows
                # [j0, j0+js), staged to DRAM in natural layout.
                for jsub in range(js // P):
                    jj = j0 + jsub * P
                    for kc in range(NKC):
                        c0 = kc * KCH
                        win = ws32.tile([P, KCH], f32, name="win")
                        nc.sync.dma_start(
                            out=win[:], in_=w[jj : jj + P, c0 : c0 + KCH]
                        )
                        a = wsg.tile([P, KCH], fp16, name="wpos")
                        nc.scalar.activation(
                            out=a[:], in_=win[:], func=act.Sign, bias=neg1[:]
                        )
                        b2 = wsg.tile([P, KCH], fp16, name="wneg")
                        nc.scalar.activation(
                            out=b2[:], in_=win[:], func=act.Sign, bias=1.0
                        )
                        t = ws16.tile([P, KCH], fp16, name="wtern")
                        nc.vector.tensor_add(out=t[:], in0=a[:], in1=b2[:])
                        nc.sync.dma_start(
                            out=wt_blocks[jj // WBLK][
                                jj % WBLK : jj % WBLK + P, c0 : c0 + KCH
                            ],
                            in_=t[:],
                        )

            def wt_t_load(wt, j0, js):
                # transpose-load W''^T rows [j0, j0+js) into wt[:, :, 0:js]
                assert j0 % WBLK == 0 and js % WBLK == 0
                for i in range(js // WBLK):
                    nc.sync.dma_start_transpose(
                        out=wt[:, :, i * WBLK : (i + 1) * WBLK],
                        in_=wt_blocks[j0 // WBLK + i][:],
                    )

            for rep in range(reps):
                if rep == 0:
                    # slab 0 W pipeline fills while X-prep runs
                    tern_slab_rows(0, JS)

                # ---- X prep: PE-transpose 128x128 f32 tiles into the psum
                # banks (TG k-tiles per bank-pass), ACT-cast to fp16 k-major
                # resident tiles. psum banks are reused as matmul
                # accumulators afterwards (deps serialize correctly).
                psums = [
                    pp.tile([P, TG, P], f32, name=f"acc{b}", bufs=1)
                    for b in range(max(BT, min(NGRP, 8)))
                ]
                NPS = len(psums)
                xtc = [
                    xtres.tile([P, KT, P], fp16, name=f"xtc{bsub}")
                    for bsub in range(BT)
                ]
                for bsub in range(BT):
                    if bench_skip_xprep and rep > 0:
                        break
                    r0 = bsub * P
                    xin = xs32.tile([P, K], f32, name="xin")
                    nc.sync.dma_start(out=xin[:], in_=x[r0 : r0 + P, :])
                    for g in range(NGRP):
                        tp = psums[(bsub + g) % NPS]
                        for i in range(TG):
                            kt = g * TG + i
                            nc.tensor.transpose(
                                tp[:, i, :], xin[:, kt * P : (kt + 1) * P],
                                ident[:],
                            )
                        nc.scalar.copy(
                            out=xtc[bsub][:, g * TG : (g + 1) * TG, :],
                            in_=tp[:],
                        )

                # ---- slab loop: ternarize (pipelined ahead), transpose-load
                # W''^T, matmul single fp16 pass, threshold, store ----
                for s in range(NSLAB):
                    j0, js = s * JS, JS
                    if not (s == 0 and rep == 0) and not (
                        bench_skip_wprep and rep > 0
                    ):
                        tern_slab_rows(j0, js)

                    wt = wtp.tile([P, KT, JS], fp16, name="wt")
                    wt_t_load(wt, j0, js)

                    if bench_skip_mm and rep > 0:
                        continue
                    for k in range(KT):
                        for b in range(BT):
                            acc = psums[b % NPS]
                            nc.tensor.matmul(
                                acc[:, :, :],
                                xtc[b][:, k, :],
                                wt[:, k, 0:js],
                                start=(k == 0),
                                stop=(k == KT - 1),
                            )
                    for b in range(BT):
                        acc = psums[b % NPS]
                        spk = op.tile([P, TG, P], fp16, name="spk")
                        nc.vector.tensor_scalar(
                            out=spk[:], in0=acc[:], scalar1=2.0,
                            scalar2=None, op0=alu.is_ge,
                        )
                        r0 = b * P
                        odma.dma_start(
                            out=out[r0 : r0 + P, j0 : j0 + js],
                            in_=spk[:],
                        )

    nc.compile()
    return nc


def _get_built():
    global _BUILT
    if _BUILT is None:
        _BUILT = build_bass(B_CORE, IN_F, OUT_F)
    return _BUILT


def kernel(spike_input: np.ndarray, synapse_states: np.ndarray) -> np.ndarray:
    from concourse.bass_utils import run_bass_kernel_spmd

    nc = _get_built()
    xs = np.ascontiguousarray(spike_input, dtype=np.float32)
    ws = np.ascontiguousarray(synapse_states, dtype=np.float32)
    in_maps = [
        {"x": xs[c * B_CORE : (c + 1) * B_CORE], "w": ws} for c in range(N_CORES)
    ]
    res = run_bass_kernel_spmd(nc, in_maps, core_ids=list(range(N_CORES)))
    out = np.empty((BATCH, OUT_F), dtype=np.float32)
    for c in range(N_CORES):
        out[c * B_CORE : (c + 1) * B_CORE] = res.results[c]["out"].astype(
            np.float32
        )
    return out
